# revision 42
# baseline (speedup 1.0000x reference)
"""Sparse (chunked-causal | bidirectional-block) GQA attention on 8 trn2 cores.

Full inputs in, full output out. Sharding: core j handles batch b = j // 4 and
kv-heads {2*(j%4), 2*(j%4)+1} (= query heads 4*(j%4) .. 4*(j%4)+3).

Split of work:
  - The DEVICE computes attention over the static chunk-causal block
    structure (all 128x128 blocks (t, s) with kv-tile t <= q-tile s in the
    same chunk). Diagonal blocks are masked with batch-exact 0/1 masks
    (causal triangle + bidirectional-run extras inside the tile);
    off-diagonal in-chunk blocks are always fully allowed.
  - Bidirectional runs that CROSS a 128-row tile boundary create a few
    extra, nearly-empty blocks off that structure. The q columns they
    touch (a handful per batch) are recomputed exactly on the HOST in
    fp32 and overwritten in the output.
  - The final softmax division also happens on the host: the device ships
    numerator rows and the denominator column (from a ones-column in V).

The host does all layout work so the device kernel is pure attention math
on DMA-friendly layouts (every descriptor is >=3KB contiguous per
partition, and the whole input is 2 tensors / 5 DMAs since same-queue DMAs
serialize):
  - qkT: K^T and Q^T (fp16, q pre-scaled by 1/sqrt(D)) interleaved and
    blocked by q-group so each group's slab is one DMA that lands just
    before its work items run.
  - vm: V in [kv%128, tile, head, d] layout with a ones column appended
    (denominators fall out of the PV matmul), plus the packed diag masks.

Per-core bass kernel, per (head, group-of-512-q) work item:
  - S^T[kv, q] via 4 PE matmuls (lhsT = K^T tile, rhs = Q^T cols), one per
    kv-tile covering [t*128, group end), first-fit packed so no piece
    crosses a PSUM bank; every matmul is a full 128-partition
    tile_position=(0,0) op (uniform PE config, weight loads pipeline).
  - one ACT exp per item -> E (fp16, SBUF).
  - DVE multiplies copy the masked diagonal blocks into a separate e2
    tile, so full-block PV matmuls depend only on exp, not the mask.
  - PV: per block, accumulate matmul lhsT=E-slice, rhs=V_aug tile into a
    2-bank PSUM group; PV matmuls of lagged items are interleaved between
    QK matmuls of the current item (hides PV weight loads), with the
    first items emitted back-free while the input DMAs stream.
  - DVE copies numerator+denominator to a head-major out tile; each
    512-q-row group stores with two half-DMAs on the scalar queue (the
    first fires after head 1, overlapping the remaining compute).
"""

import math

import numpy as np

import concourse.bass as bass
import concourse.mybir as mybir
import concourse.tile as tile
from concourse import bacc
from concourse.bass_utils import run_bass_kernel_spmd

B, S, HQ, HKV, D = 2, 2048, 16, 8, 128
TS = 128                  # block tile size (partitions)
NT = S // TS              # 16 q/kv tiles
GROUP_SUBTILES = 4        # q-subtiles per group (512 q rows)
N_GROUPS = NT // GROUP_SUBTILES
BANK_COLS = 512           # fp32 cols per PSUM bank
ST_COLS = 1536            # st tile cols (3 banks; one group in one round)
N_CORES = 8
PAIRS_PER_CORE = 2        # kv heads per core
HEADS_PER_CORE = 4        # query heads per core

F16 = mybir.dt.float16
F32 = mybir.dt.float32

V_COLS = NT * PAIRS_PER_CORE * (D + 1)   # v part of the merged vm tensor
VM_COLS = None                           # set per-schedule at build time


# ---------------------------------------------------------------- host masks

def _segment_ids(m):
    """[B, S] 0/1 -> contiguous-run segment ids (0 = not in a run)."""
    mm = m.astype(np.int64)
    padded = np.pad(mm, ((0, 0), (1, 0)))
    boundary = padded[:, 1:] > padded[:, :-1]
    return mm * np.cumsum(boundary, axis=1)


def _allowed_T(bidirectional_mask, chunk):
    """Per-batch allowed mask, transposed: [B, S(kv), S(q)] bool."""
    seg = _segment_ids(np.asarray(bidirectional_mask))
    r = np.arange(S)
    chunk_ok = (r[:, None] // chunk == r[None, :] // chunk) & (r[:, None] >= r[None, :])
    out = np.zeros((B, S, S), dtype=bool)
    for b in range(B):
        bid = (seg[b][:, None] == seg[b][None, :]) & (seg[b][:, None] > 0)
        out[b] = (chunk_ok | bid).T
    return out


class Schedule:
    """Device schedule over the static chunk-causal structure; any u_any
    block off that structure is deferred to the host (fix_cols).

    groups[g] = dict with fields:
      cols: total packed e-columns
      qk:   [(t, e_off, q_abs, n)]   matmul pieces, none crossing a bank
      mask: (e_lo, mbuf_off, w)      single DVE mask mult (diag tail)
      pv:   {s_local: [(t, e_off)]}  accumulation lists (all 128-wide)
    """

    def __init__(self, allowed_T, chunk):
        blocks = allowed_T.reshape(B, NT, TS, NT, TS)
        b_any = blocks.any(axis=(2, 4))
        u_any = b_any.any(axis=0)
        tpc = max(chunk // TS, 1)   # tiles per chunk
        tt, ss = np.meshgrid(np.arange(NT), np.arange(NT), indexing="ij")
        causal = (tt // tpc == ss // tpc) & (ss >= tt)

        # host-fix columns: q extents of any allowed block off the structure
        colmask = blocks.any(axis=(0, 2))  # [t, s, q_in_tile]
        fix = np.zeros(S, dtype=bool)
        for t in range(NT):
            for s in range(NT):
                if u_any[t, s] and not causal[t, s]:
                    fix[s * TS:(s + 1) * TS] |= colmask[t, s]
        self.fix_cols = np.nonzero(fix)[0]

        self.mask_slices = []   # ordered t of diag blocks -> host buffer
        mbuf_off = 0
        self.groups = []
        for g in range(N_GROUPS):
            s0 = g * GROUP_SUBTILES
            t_list = [t for t in range(NT)
                      if any(causal[t, s] for s in range(s0, s0 + GROUP_SUBTILES))]
            # full pieces: for t, span of s>t blocks in group; diag last
            fulls = []
            for t in t_list:
                ss_full = [s for s in range(s0, s0 + GROUP_SUBTILES)
                           if causal[t, s] and s != t]
                if ss_full:
                    lo, hi = min(ss_full), max(ss_full) + 1
                    assert ss_full == list(range(lo, hi))
                    fulls.append((t, lo, hi - lo))
            diags = [t for t in t_list if s0 <= t < s0 + GROUP_SUBTILES]

            work = {"cols": 0, "qk": [], "masks": [],
                    "pv": {sl: [] for sl in range(GROUP_SUBTILES)}}
            e_of_block = {}
            entries = []   # one piece per t: q range [t*TS, group end)
            for t in diags:
                entries.append((t, (s0 + GROUP_SUBTILES) * TS - t * TS))
            for (t, lo, nsub) in fulls:
                if t not in diags:
                    entries.append((t, nsub * TS))
            # first-fit-decreasing into 512-col banks (no piece crosses one)
            entries.sort(key=lambda x: -x[1])
            off = 0
            placed = []
            rem = list(entries)
            while rem:
                pick = None
                for idx, (t, w) in enumerate(rem):
                    room = BANK_COLS - off % BANK_COLS
                    if w <= room or off % BANK_COLS == 0:
                        pick = idx
                        break
                if pick is None:
                    off += BANK_COLS - off % BANK_COLS
                    continue
                t, w = rem.pop(pick)
                q_abs = t * TS if t in diags else                     (min(s for s in range(s0, s0 + GROUP_SUBTILES)
                         if causal[t, s] and s != t)) * TS
                work["qk"].append((t, off, q_abs, w))
                base_s = q_abs // TS
                for i in range(w // TS):
                    e_of_block[(t, base_s + i)] = off + i * TS
                placed.append((t, off, w))
                off += w
            work["cols"] = off
            assert off <= ST_COLS, f"group {g}: {off} cols > {ST_COLS}"

            # masks: merge adjacent diag spans; e2 packs them in op order
            diag_offs = sorted((e_of_block[(t, t)], t) for t in diags)
            e2_off = 0
            for (eo, t) in diag_offs:
                if work["masks"] and                         work["masks"][-1][0] + work["masks"][-1][2] == eo:
                    work["masks"][-1][2] += TS
                else:
                    work["masks"].append([eo, mbuf_off + e2_off, TS])
                e_of_block[(t, t)] = ("e2", e2_off)
                self.mask_slices.append(t)
                e2_off += TS
            work["masks"] = [tuple(x) for x in work["masks"]]
            work["e2_cols"] = e2_off
            mbuf_off += e2_off

            for s in range(s0, s0 + GROUP_SUBTILES):
                for t in range(NT):
                    if causal[t, s]:
                        work["pv"][s - s0].append((t, e_of_block[(t, s)]))
            self.groups.append(work)

        self.n_mask_cols = mbuf_off

    def mask_data(self, allowed_T_b):
        """[TS, n_mask_cols] fp16 0/1 packed diag-mask buffer, one batch."""
        out = np.zeros((TS, max(self.n_mask_cols, 1)), dtype=np.float16)
        for i, t in enumerate(self.mask_slices):
            out[:, i * TS:(i + 1) * TS] = \
                allowed_T_b[t * TS:(t + 1) * TS, t * TS:(t + 1) * TS]
        return out

    def key(self):
        return (tuple(self.mask_slices),
                tuple((g["cols"], tuple(g["qk"])) for g in self.groups))


# ------------------------------------------------------------- kernel build

def _broadcast_free(ap, n):
    """Append a 0-step free dim of size n to an AP (read-broadcast)."""
    return bass.AP(tensor=ap.tensor, offset=ap.offset, ap=[*ap.ap, [0, n]])


def _split_dim(ap, n0, n1):
    """Split an AP's first free dim of size n0*n1 into (n0, n1)."""
    (pstep, pnum), (fstep, fnum), *rest = ap.ap
    assert fnum == n0 * n1
    return bass.AP(tensor=ap.tensor, offset=ap.offset,
                   ap=[[pstep, pnum], [fstep * n1, n0], [fstep, n1], *rest])


def _build_body(nc, tc, sched: Schedule, tensors, safe_pv=False):
    qk_in, vm_in, o_out = tensors
    ctxs = []
    pv_first_mms = []

    def pool(*a, **kw):
        p = tc.tile_pool(*a, **kw)
        ctxs.append(p)
        return p.__enter__()

    ktp = pool(name="ktp", bufs=N_GROUPS + 1)
    vp = pool(name="vp", bufs=1)
    epool = pool(name="epool", bufs=6)
    e2pool = pool(name="e2pool", bufs=6)
    outp = pool(name="outp", bufs=N_GROUPS)
    stp = pool(name="st_psum", bufs=1 if safe_pv else 2, space="PSUM")
    pvp = pool(name="pv_psum", bufs=1, space="PSUM")

    # Inputs are merged host-side into two tensors; qk is group-blocked
    # ([TS, group, 6 heads, 512 cols]) so each group's slab is one
    # full-speed contiguous DMA that lands just before its items run.
    QS = S // 4
    qk_sb = [None] * N_GROUPS
    qk_a0 = ktp.tile([TS, 3, QS], F16, name="qk_a0", tag="qk0")
    nc.sync.dma_start(out=qk_a0, in_=qk_in[:, 0, 0:3, :])
    qk_a1 = ktp.tile([TS, 3, QS], F16, name="qk_a1", tag="qk0")
    nc.scalar.dma_start(out=qk_a1, in_=qk_in[:, 0, 3:6, :])
    vm_sb = vp.tile([TS, VM_COLS], F16, name="vm_sb", tag="vm")
    nc.scalar.dma_start(out=vm_sb, in_=vm_in[:, :])
    for g_ in range(1, N_GROUPS):
        qk_sb[g_] = ktp.tile([TS, 6, QS], F16, name=f"qk_g{g_}", tag="qk")
        eng = nc.scalar if g_ == 2 else nc.sync
        eng.dma_start(out=qk_sb[g_], in_=qk_in[:, g_, :, :])

    def _qk_slice(hi, q0, n):
        g_, o = divmod(q0, QS)
        assert o + n <= QS
        if g_ == 0:
            t_ = qk_a0 if hi < 3 else qk_a1
            return t_[:, hi % 3, o:o + n]
        return qk_sb[g_][:, hi, o:o + n]

    def kt_slice(pair, t):
        return _qk_slice(pair, t * TS, TS)

    def qt_slice(head, q0, n):
        return _qk_slice(2 + head, q0, n)

    def v_slice(t, pair):
        return vm_sb[:, (t * PAIRS_PER_CORE + pair) * (D + 1):
                     (t * PAIRS_PER_CORE + pair) * (D + 1) + D + 1]

    def mask_slice(moff, mw):
        return vm_sb[:, V_COLS + moff:V_COLS + moff + mw]

    out_tiles = [outp.tile([TS, HEADS_PER_CORE, GROUP_SUBTILES, D + 1], F16,
                           name=f"out_{g}", tag="out")
                 for g in range(N_GROUPS)]

    nbank = GROUP_SUBTILES if safe_pv else 2
    per = 1 if safe_pv else 2

    work = []
    for g in range(N_GROUPS):
        for head in range(HEADS_PER_CORE):
            work.append({"head": head, "pair": head // 2, "g": g,
                         "w": sched.groups[g]})

    def front_mms(w):
        gw = w["w"]
        st = stp.tile([TS, ST_COLS], F32, tag="st")
        w["st"] = st
        thunks = []
        for (t, e_off, q0, n) in gw["qk"]:
            def mk(t=t, e_off=e_off, q0=q0, n=n):
                nc.tensor.matmul(
                    st[:, e_off:e_off + n],
                    lhsT=kt_slice(w["pair"], t),
                    rhs=qt_slice(w["head"], q0, n),
                    start=True, stop=True,
                )
            thunks.append(mk)
        return thunks

    def front_tail(w):
        gw = w["w"]
        st = w["st"]
        e = epool.tile([TS, ST_COLS], F16, tag="e")
        nc.scalar.activation(
            e[:, 0:gw["cols"]], st[:, 0:gw["cols"]],
            mybir.ActivationFunctionType.Exp,
        )
        w["e"] = e
        w["e2"] = None
        if gw["masks"]:
            e2 = e2pool.tile([TS, BANK_COLS], F16, tag="e2")
            e2_off = 0
            m0 = gw["masks"][0][1]
            for (e_lo, moff, mw) in gw["masks"]:
                nc.vector.tensor_mul(
                    e2[:, moff - m0:moff - m0 + mw],
                    e[:, e_lo:e_lo + mw],
                    mask_slice(moff, mw),
                )
            w["e2"] = e2

    def back_mms(w):
        gw, g, head, pair = w["w"], w["g"], w["head"], w["pair"]
        pv = pvp.tile([TS, nbank, per, BANK_COLS // per], F32,
                      name=f"pv_{head}_{g}", tag="pv")
        w["pv"] = pv
        e = w["e"]
        bank_first = [None] * nbank
        bank_mms = [[] for _ in range(nbank)]
        bank_total = [0] * nbank
        bank_done = [0] * nbank
        for sl in range(GROUP_SUBTILES):
            bank_total[sl // per] += len(gw["pv"][sl])
        full_thunks, diag_thunks = [], []
        for sl in range(GROUP_SUBTILES):
            bk, sub = divmod(sl, per)
            for (t, e_off) in gw["pv"][sl]:
                diag = isinstance(e_off, tuple)

                def mk(bk=bk, sub=sub, t=t, e_off=e_off, diag=diag):
                    src_ = (w["e2"][:, e_off[1]:e_off[1] + TS] if diag
                            else e[:, e_off:e_off + TS])
                    first = bank_first[bk] is None
                    bank_done[bk] += 1
                    mm = nc.tensor.matmul(
                        pv[:, bk, sub, 0:D + 1],
                        lhsT=src_,
                        rhs=v_slice(t, pair),
                        start=first,
                        stop=bank_done[bk] == bank_total[bk],
                    )
                    if first:
                        bank_first[bk] = mm.ins.name
                    else:
                        bank_mms[bk].append(mm.ins.name)
                (diag_thunks if diag else full_thunks).append(mk)
        w["bank_state"] = (bank_first, bank_mms)
        return full_thunks + diag_thunks

    def back_tail(w):
        g, head = w["g"], w["head"]
        pv = w["pv"]
        (bank_first, bank_mms) = w["bank_state"]
        pv_first_mms.extend(
            (f, o) for f, o in zip(bank_first, bank_mms) if f is not None)
        # ship numerator + denominator; the host divides
        out_t = out_tiles[g]
        out_ap = _split_dim(out_t[:, head, :, :], nbank, per)
        nc.vector.tensor_copy(out_ap, pv[:, :, :, 0:D + 1])
        if head == 1:
            nc.scalar.dma_start(out=o_out[:, g, 0:2, :, :],
                                in_=out_t[:, 0:2, :, :])
        elif head == HEADS_PER_CORE - 1:
            nc.scalar.dma_start(out=o_out[:, g, 2:4, :, :],
                                in_=out_t[:, 2:4, :, :])

    def interleave(a, b):
        if not b:
            return list(a)
        if not a:
            return list(b)
        out = []
        na, nb = len(a), len(b)
        ia = ib = 0
        while ia < na or ib < nb:
            if ia < na:
                out.append(a[ia])
                ia += 1
            while ib * na <= ia * nb and ib < nb:
                out.append(b[ib])
                ib += 1
        return out

    # Fronts 0..EARLY-1 run without interleaved backs: the PV of item 0
    # needs the vm tensor, which is still loading while group 0's QK work
    # is already possible; an interleaved (in-order) PE queue would stall.
    n = len(work)
    EARLY = min(5, n)
    LAG = 2
    backlog = list(range(n))   # items whose back phase is pending
    done_front = 0
    for i in range(n):
        take = []
        if i >= EARLY:
            want = len(backlog) - (n - 1 - i)   # drain so last front pairs last back
            want = max(want, 1 if backlog and backlog[0] <= i - LAG else 0)
            for _ in range(min(want, 2)):
                if backlog and backlog[0] <= i - LAG:
                    take.append(backlog.pop(0))
        fr = front_mms(work[i])
        bks = []
        for j in take:
            bks.extend(back_mms(work[j]))
        for thunk in interleave(fr, bks):
            thunk()
        for j in take:
            back_tail(work[j])
        front_tail(work[i])
    while backlog:
        j = backlog.pop(0)
        for thunk in back_mms(work[j]):
            thunk()
        back_tail(work[j])

    for p in reversed(ctxs):
        p.__exit__(None, None, None)
    return pv_first_mms


def _verify_pv_order(nc, pv_first_mms):
    pos = {}
    i = 0
    for bb in nc.m.functions[0].blocks:
        for ins in bb.instructions:
            pos[ins.name] = i
            i += 1
    for first, others in pv_first_mms:
        p0 = pos.get(first)
        if p0 is None:
            return False
        for o in others:
            po = pos.get(o)
            if po is None or po < p0:
                return False
    return True


def _build_kernel(sched: Schedule, safe_pv: bool = False):
    global VM_COLS
    VM_COLS = V_COLS + max(sched.n_mask_cols, 1)
    nc = bacc.Bacc("TRN2", target_bir_lowering=False, debug=False,
                   num_devices=N_CORES, name="sparse_attn")

    qk_in = nc.dram_tensor("qkT", [TS, N_GROUPS, 6, S // N_GROUPS], F16, kind="ExternalInput")
    vm_in = nc.dram_tensor("vm", [TS, VM_COLS], F16, kind="ExternalInput")
    o_out = nc.dram_tensor("o", [TS, N_GROUPS, HEADS_PER_CORE, GROUP_SUBTILES, D + 1],
                           F16, kind="ExternalOutput")
    tensors = (qk_in, vm_in, o_out)

    with tile.TileContext(nc) as tc:
        pv_first_mms = _build_body(nc, tc, sched, tensors, safe_pv=safe_pv)

    nc.compile()
    if not safe_pv and not _verify_pv_order(nc, pv_first_mms):
        return _build_kernel(sched, safe_pv=True)
    return nc


# --------------------------------------------------------------- entry point

_CACHE = {}


def _get_kernel(sched: Schedule):
    key = sched.key()
    if key not in _CACHE:
        _CACHE[key] = _build_kernel(sched)
    return _CACHE[key]


def _shard_inputs(q, k, v, masks_f16, n_mask_cols):
    scale = 1.0 / math.sqrt(D)
    vm_cols = V_COLS + max(n_mask_cols, 1)
    in_maps = []
    for core in range(N_CORES):
        b = core // 4
        m = core % 4
        qk6 = np.empty((TS, 6, S), dtype=np.float16)
        qk6[:, 0:2, :] = k[b, :, 2 * m:2 * m + 2, :].astype(np.float16).transpose(2, 1, 0)
        qk6[:, 2:6, :] = (q[b, :, 4 * m:4 * m + 4, :] * scale).astype(np.float16).transpose(2, 1, 0)
        qkT = np.ascontiguousarray(
            qk6.reshape(TS, 6, N_GROUPS, S // N_GROUPS).transpose(0, 2, 1, 3))
        vc = v[b, :, 2 * m:2 * m + 2, :].astype(np.float16)
        vaug = np.ones((S, 2, D + 1), dtype=np.float16)
        vaug[:, :, :D] = vc
        vaug = vaug.reshape(NT, TS, 2, D + 1).transpose(1, 0, 2, 3)
        vm = np.zeros((TS, vm_cols), dtype=np.float16)
        vm[:, 0:V_COLS] = vaug.reshape(TS, V_COLS)
        vm[:, V_COLS:V_COLS + masks_f16[b].shape[1]] = masks_f16[b]
        in_maps.append({"qkT": qkT, "vm": vm})
    return in_maps


def _host_fix(out, q, k, v, allowed_T, cols):
    """Recompute the given q columns exactly (fp32) and overwrite."""
    if len(cols) == 0:
        return
    scale = 1.0 / math.sqrt(D)
    group = HQ // HKV
    for b in range(B):
        qb = q[b, cols, :, :]                          # [R, HQ, D]
        al = allowed_T[b][:, cols].T                   # [R, S(kv)]
        # logits[r, hq, kv]
        kb = np.repeat(k[b], group, axis=1)            # [S, HQ, D]
        logits = np.einsum("rhd,shd->rhs", qb * scale, kb)
        logits = np.where(al[:, None, :], logits, -np.inf)
        mx = logits.max(axis=-1, keepdims=True)
        e = np.exp(logits - mx)
        p = e / e.sum(axis=-1, keepdims=True)
        vb = np.repeat(v[b], group, axis=1)            # [S, HQ, D]
        out[b, cols, :, :] = np.einsum("rhs,shd->rhd", p, vb)


def kernel(q, k, v, bidirectional_mask, chunk_size):
    q = np.asarray(q, dtype=np.float32)
    k = np.asarray(k, dtype=np.float32)
    v = np.asarray(v, dtype=np.float32)
    chunk = int(np.asarray(chunk_size))

    allowed_T = _allowed_T(bidirectional_mask, chunk)
    sched = Schedule(allowed_T, chunk)
    nc = _get_kernel(sched)

    masks_f16 = [sched.mask_data(allowed_T[b]) for b in range(B)]
    in_maps = _shard_inputs(q, k, v, masks_f16, sched.n_mask_cols)

    res = run_bass_kernel_spmd(nc, in_maps, list(range(N_CORES)))

    out = np.empty((B, S, HQ, D), dtype=np.float32)
    for core in range(N_CORES):
        b = core // 4
        m = core % 4
        oc = res.results[core]["o"]     # [TS, N_GROUPS, 4, GROUP_SUBTILES, D+1]
        oc = oc.transpose(1, 3, 0, 2, 4).reshape(S, HEADS_PER_CORE, D + 1)
        oc = oc.astype(np.float32)
        out[b, :, 4 * m:4 * m + 4, :] = oc[:, :, :D] / oc[:, :, D:]

    _host_fix(out, q, k, v, allowed_T, sched.fix_cols)
    return out


# revision 43
# speedup vs baseline: 1.0205x; 1.0205x over previous
"""Sparse (chunked-causal | bidirectional-block) GQA attention on 8 trn2 cores.

Full inputs in, full output out. Sharding: core j handles batch b = j // 4 and
kv-heads {2*(j%4), 2*(j%4)+1} (= query heads 4*(j%4) .. 4*(j%4)+3).

Split of work:
  - The DEVICE computes attention over the static chunk-causal block
    structure (all 128x128 blocks (t, s) with kv-tile t <= q-tile s in the
    same chunk). Diagonal blocks are masked with batch-exact 0/1 masks
    (causal triangle + bidirectional-run extras inside the tile);
    off-diagonal in-chunk blocks are always fully allowed.
  - Bidirectional runs that CROSS a 128-row tile boundary create a few
    extra, nearly-empty blocks off that structure. The q columns they
    touch (a handful per batch) are recomputed exactly on the HOST in
    fp32 and overwritten in the output.
  - The final softmax division also happens on the host: the device ships
    numerator rows and the denominator column (from a ones-column in V).

The host does all layout work so the device kernel is pure attention math
on DMA-friendly layouts (every descriptor is >=3KB contiguous per
partition, and the whole input is 2 tensors / 5 DMAs since same-queue DMAs
serialize):
  - qkT: K^T and Q^T (fp16, q pre-scaled by 1/sqrt(D)) interleaved and
    blocked by q-group so each group's slab is one DMA that lands just
    before its work items run.
  - vm: V in [kv%128, tile, head, d] layout with a ones column appended
    (denominators fall out of the PV matmul), plus the packed diag masks.

Per-core bass kernel, per (head, group-of-512-q) work item:
  - S^T[kv, q] via 4 PE matmuls (lhsT = K^T tile, rhs = Q^T cols), one per
    kv-tile covering [t*128, group end), first-fit packed so no piece
    crosses a PSUM bank; every matmul is a full 128-partition
    tile_position=(0,0) op (uniform PE config, weight loads pipeline).
  - one ACT exp per item -> E (fp16, SBUF).
  - DVE multiplies copy the masked diagonal blocks into a separate e2
    tile, so full-block PV matmuls depend only on exp, not the mask.
  - PV: per block, accumulate matmul lhsT=E-slice, rhs=V_aug tile into a
    2-bank PSUM group; PV matmuls of lagged items are interleaved between
    QK matmuls of the current item (hides PV weight loads), with the
    first items emitted back-free while the input DMAs stream.
  - DVE copies numerator+denominator to a head-major out tile; each
    512-q-row group stores with two half-DMAs on the scalar queue (the
    first fires after head 1, overlapping the remaining compute).
"""

import math

import numpy as np

import concourse.bass as bass
import concourse.mybir as mybir
import concourse.tile as tile
from concourse import bacc
from concourse.bass_utils import run_bass_kernel_spmd

B, S, HQ, HKV, D = 2, 2048, 16, 8, 128
TS = 128                  # block tile size (partitions)
NT = S // TS              # 16 q/kv tiles
GROUP_SUBTILES = 4        # q-subtiles per group (512 q rows)
N_GROUPS = NT // GROUP_SUBTILES
BANK_COLS = 512           # fp32 cols per PSUM bank
ST_COLS = 1536            # st tile cols (3 banks; one group in one round)
N_CORES = 8
PAIRS_PER_CORE = 2        # kv heads per core
HEADS_PER_CORE = 4        # query heads per core

F16 = mybir.dt.float16
F32 = mybir.dt.float32

V_COLS = NT * PAIRS_PER_CORE * (D + 1)   # v part of the merged vm tensor
VM_COLS = None                           # set per-schedule at build time


# ---------------------------------------------------------------- host masks

def _segment_ids(m):
    """[B, S] 0/1 -> contiguous-run segment ids (0 = not in a run)."""
    mm = m.astype(np.int64)
    padded = np.pad(mm, ((0, 0), (1, 0)))
    boundary = padded[:, 1:] > padded[:, :-1]
    return mm * np.cumsum(boundary, axis=1)


def _allowed_T(bidirectional_mask, chunk):
    """Per-batch allowed mask, transposed: [B, S(kv), S(q)] bool."""
    seg = _segment_ids(np.asarray(bidirectional_mask))
    r = np.arange(S)
    chunk_ok = (r[:, None] // chunk == r[None, :] // chunk) & (r[:, None] >= r[None, :])
    out = np.zeros((B, S, S), dtype=bool)
    for b in range(B):
        bid = (seg[b][:, None] == seg[b][None, :]) & (seg[b][:, None] > 0)
        out[b] = (chunk_ok | bid).T
    return out


class Schedule:
    """Device schedule over the static chunk-causal structure; any u_any
    block off that structure is deferred to the host (fix_cols).

    groups[g] = dict with fields:
      cols: total packed e-columns
      qk:   [(t, e_off, q_abs, n)]   matmul pieces, none crossing a bank
      mask: (e_lo, mbuf_off, w)      single DVE mask mult (diag tail)
      pv:   {s_local: [(t, e_off)]}  accumulation lists (all 128-wide)
    """

    def __init__(self, allowed_T, chunk):
        blocks = allowed_T.reshape(B, NT, TS, NT, TS)
        b_any = blocks.any(axis=(2, 4))
        u_any = b_any.any(axis=0)
        tpc = max(chunk // TS, 1)   # tiles per chunk
        tt, ss = np.meshgrid(np.arange(NT), np.arange(NT), indexing="ij")
        causal = (tt // tpc == ss // tpc) & (ss >= tt)

        # host-fix columns: q extents of any allowed block off the structure
        colmask = blocks.any(axis=(0, 2))  # [t, s, q_in_tile]
        fix = np.zeros(S, dtype=bool)
        for t in range(NT):
            for s in range(NT):
                if u_any[t, s] and not causal[t, s]:
                    fix[s * TS:(s + 1) * TS] |= colmask[t, s]
        self.fix_cols = np.nonzero(fix)[0]

        self.mask_slices = []   # ordered t of diag blocks -> host buffer
        mbuf_off = 0
        self.groups = []
        for g in range(N_GROUPS):
            s0 = g * GROUP_SUBTILES
            t_list = [t for t in range(NT)
                      if any(causal[t, s] for s in range(s0, s0 + GROUP_SUBTILES))]
            # full pieces: for t, span of s>t blocks in group; diag last
            fulls = []
            for t in t_list:
                ss_full = [s for s in range(s0, s0 + GROUP_SUBTILES)
                           if causal[t, s] and s != t]
                if ss_full:
                    lo, hi = min(ss_full), max(ss_full) + 1
                    assert ss_full == list(range(lo, hi))
                    fulls.append((t, lo, hi - lo))
            diags = [t for t in t_list if s0 <= t < s0 + GROUP_SUBTILES]

            work = {"cols": 0, "qk": [], "masks": [],
                    "pv": {sl: [] for sl in range(GROUP_SUBTILES)}}
            e_of_block = {}
            entries = []   # one piece per t: q range [t*TS, group end)
            for t in diags:
                entries.append((t, (s0 + GROUP_SUBTILES) * TS - t * TS))
            for (t, lo, nsub) in fulls:
                if t not in diags:
                    entries.append((t, nsub * TS))
            # first-fit-decreasing into 512-col banks (no piece crosses one)
            entries.sort(key=lambda x: -x[1])
            off = 0
            placed = []
            rem = list(entries)
            while rem:
                pick = None
                for idx, (t, w) in enumerate(rem):
                    room = BANK_COLS - off % BANK_COLS
                    if w <= room or off % BANK_COLS == 0:
                        pick = idx
                        break
                if pick is None:
                    off += BANK_COLS - off % BANK_COLS
                    continue
                t, w = rem.pop(pick)
                q_abs = t * TS if t in diags else                     (min(s for s in range(s0, s0 + GROUP_SUBTILES)
                         if causal[t, s] and s != t)) * TS
                work["qk"].append((t, off, q_abs, w))
                base_s = q_abs // TS
                for i in range(w // TS):
                    e_of_block[(t, base_s + i)] = off + i * TS
                placed.append((t, off, w))
                off += w
            work["cols"] = off
            assert off <= ST_COLS, f"group {g}: {off} cols > {ST_COLS}"

            # masks: merge adjacent diag spans; e2 packs them in op order
            diag_offs = sorted((e_of_block[(t, t)], t) for t in diags)
            e2_off = 0
            for (eo, t) in diag_offs:
                if work["masks"] and                         work["masks"][-1][0] + work["masks"][-1][2] == eo:
                    work["masks"][-1][2] += TS
                else:
                    work["masks"].append([eo, mbuf_off + e2_off, TS])
                e_of_block[(t, t)] = ("e2", e2_off)
                self.mask_slices.append(t)
                e2_off += TS
            work["masks"] = [tuple(x) for x in work["masks"]]
            work["e2_cols"] = e2_off
            mbuf_off += e2_off

            for s in range(s0, s0 + GROUP_SUBTILES):
                for t in range(NT):
                    if causal[t, s]:
                        work["pv"][s - s0].append((t, e_of_block[(t, s)]))
            self.groups.append(work)

        self.n_mask_cols = mbuf_off

    def mask_data(self, allowed_T_b):
        """[TS, n_mask_cols] fp16 0/1 packed diag-mask buffer, one batch."""
        out = np.zeros((TS, max(self.n_mask_cols, 1)), dtype=np.float16)
        for i, t in enumerate(self.mask_slices):
            out[:, i * TS:(i + 1) * TS] = \
                allowed_T_b[t * TS:(t + 1) * TS, t * TS:(t + 1) * TS]
        return out

    def key(self):
        return (tuple(self.mask_slices),
                tuple((g["cols"], tuple(g["qk"])) for g in self.groups))


# ------------------------------------------------------------- kernel build

def _broadcast_free(ap, n):
    """Append a 0-step free dim of size n to an AP (read-broadcast)."""
    return bass.AP(tensor=ap.tensor, offset=ap.offset, ap=[*ap.ap, [0, n]])


def _split_dim(ap, n0, n1):
    """Split an AP's first free dim of size n0*n1 into (n0, n1)."""
    (pstep, pnum), (fstep, fnum), *rest = ap.ap
    assert fnum == n0 * n1
    return bass.AP(tensor=ap.tensor, offset=ap.offset,
                   ap=[[pstep, pnum], [fstep * n1, n0], [fstep, n1], *rest])


def _build_body(nc, tc, sched: Schedule, tensors, safe_pv=False):
    qk_in, vm_in, o_out = tensors
    ctxs = []
    pv_first_mms = []

    def pool(*a, **kw):
        p = tc.tile_pool(*a, **kw)
        ctxs.append(p)
        return p.__enter__()

    ktp = pool(name="ktp", bufs=N_GROUPS)
    vp = pool(name="vp", bufs=1)
    epool = pool(name="epool", bufs=6)
    e2pool = pool(name="e2pool", bufs=6)
    outp = pool(name="outp", bufs=N_GROUPS)
    stp = pool(name="st_psum", bufs=1 if safe_pv else 2, space="PSUM")
    pvp = pool(name="pv_psum", bufs=1, space="PSUM")

    # Inputs are merged host-side into two tensors; qk is group-blocked
    # ([TS, group, 6 heads, 512 cols]) so each group's slab is one
    # full-speed contiguous DMA that lands just before its items run.
    QS = S // 4
    qk_sb = [None] * N_GROUPS
    qk_sb[0] = ktp.tile([TS, 6, QS], F16, name="qk_g0", tag="qk")
    nc.sync.dma_start(out=qk_sb[0], in_=qk_in[:, 0, :, :])
    vm_sb = vp.tile([TS, VM_COLS], F16, name="vm_sb", tag="vm")
    nc.scalar.dma_start(out=vm_sb, in_=vm_in[:, :])
    for g_ in range(1, N_GROUPS):
        qk_sb[g_] = ktp.tile([TS, 6, QS], F16, name=f"qk_g{g_}", tag="qk")
        nc.sync.dma_start(out=qk_sb[g_], in_=qk_in[:, g_, :, :])

    def _qk_slice(hi, q0, n):
        g_, o = divmod(q0, QS)
        assert o + n <= QS
        return qk_sb[g_][:, hi, o:o + n]

    def kt_slice(pair, t):
        return _qk_slice(pair, t * TS, TS)

    def qt_slice(head, q0, n):
        return _qk_slice(2 + head, q0, n)

    def v_slice(t, pair):
        return vm_sb[:, (t * PAIRS_PER_CORE + pair) * (D + 1):
                     (t * PAIRS_PER_CORE + pair) * (D + 1) + D + 1]

    def mask_slice(moff, mw):
        return vm_sb[:, V_COLS + moff:V_COLS + moff + mw]

    out_tiles = [outp.tile([TS, HEADS_PER_CORE, GROUP_SUBTILES, D + 1], F16,
                           name=f"out_{g}", tag="out")
                 for g in range(N_GROUPS)]

    nbank = GROUP_SUBTILES if safe_pv else 2
    per = 1 if safe_pv else 2

    work = []
    for g in range(N_GROUPS):
        for head in range(HEADS_PER_CORE):
            work.append({"head": head, "pair": head // 2, "g": g,
                         "w": sched.groups[g]})

    def front_mms(w):
        gw = w["w"]
        st = stp.tile([TS, ST_COLS], F32, tag="st")
        w["st"] = st
        thunks = []
        for (t, e_off, q0, n) in gw["qk"]:
            def mk(t=t, e_off=e_off, q0=q0, n=n):
                nc.tensor.matmul(
                    st[:, e_off:e_off + n],
                    lhsT=kt_slice(w["pair"], t),
                    rhs=qt_slice(w["head"], q0, n),
                    start=True, stop=True,
                )
            thunks.append(mk)
        return thunks

    def front_tail(w):
        gw = w["w"]
        st = w["st"]
        e = epool.tile([TS, ST_COLS], F16, tag="e")
        nc.scalar.activation(
            e[:, 0:gw["cols"]], st[:, 0:gw["cols"]],
            mybir.ActivationFunctionType.Exp,
        )
        w["e"] = e
        w["e2"] = None
        if gw["masks"]:
            e2 = e2pool.tile([TS, BANK_COLS], F16, tag="e2")
            e2_off = 0
            m0 = gw["masks"][0][1]
            for (e_lo, moff, mw) in gw["masks"]:
                nc.vector.tensor_mul(
                    e2[:, moff - m0:moff - m0 + mw],
                    e[:, e_lo:e_lo + mw],
                    mask_slice(moff, mw),
                )
            w["e2"] = e2

    def back_mms(w):
        gw, g, head, pair = w["w"], w["g"], w["head"], w["pair"]
        pv = pvp.tile([TS, nbank, per, BANK_COLS // per], F32,
                      name=f"pv_{head}_{g}", tag="pv")
        w["pv"] = pv
        e = w["e"]
        bank_first = [None] * nbank
        bank_mms = [[] for _ in range(nbank)]
        bank_total = [0] * nbank
        bank_done = [0] * nbank
        for sl in range(GROUP_SUBTILES):
            bank_total[sl // per] += len(gw["pv"][sl])
        full_thunks, diag_thunks = [], []
        for sl in range(GROUP_SUBTILES):
            bk, sub = divmod(sl, per)
            for (t, e_off) in gw["pv"][sl]:
                diag = isinstance(e_off, tuple)

                def mk(bk=bk, sub=sub, t=t, e_off=e_off, diag=diag):
                    src_ = (w["e2"][:, e_off[1]:e_off[1] + TS] if diag
                            else e[:, e_off:e_off + TS])
                    first = bank_first[bk] is None
                    bank_done[bk] += 1
                    mm = nc.tensor.matmul(
                        pv[:, bk, sub, 0:D + 1],
                        lhsT=src_,
                        rhs=v_slice(t, pair),
                        start=first,
                        stop=bank_done[bk] == bank_total[bk],
                    )
                    if first:
                        bank_first[bk] = mm.ins.name
                    else:
                        bank_mms[bk].append(mm.ins.name)
                (diag_thunks if diag else full_thunks).append(mk)
        w["bank_state"] = (bank_first, bank_mms)
        return full_thunks + diag_thunks

    def back_tail(w):
        g, head = w["g"], w["head"]
        pv = w["pv"]
        (bank_first, bank_mms) = w["bank_state"]
        pv_first_mms.extend(
            (f, o) for f, o in zip(bank_first, bank_mms) if f is not None)
        # ship numerator + denominator; the host divides
        out_t = out_tiles[g]
        out_ap = _split_dim(out_t[:, head, :, :], nbank, per)
        nc.vector.tensor_copy(out_ap, pv[:, :, :, 0:D + 1])
        if head == 1:
            nc.scalar.dma_start(out=o_out[:, g, 0:2, :, :],
                                in_=out_t[:, 0:2, :, :])
        elif head == HEADS_PER_CORE - 1:
            nc.scalar.dma_start(out=o_out[:, g, 2:4, :, :],
                                in_=out_t[:, 2:4, :, :])

    def interleave(a, b):
        if not b:
            return list(a)
        if not a:
            return list(b)
        out = []
        na, nb = len(a), len(b)
        ia = ib = 0
        while ia < na or ib < nb:
            if ia < na:
                out.append(a[ia])
                ia += 1
            while ib * na <= ia * nb and ib < nb:
                out.append(b[ib])
                ib += 1
        return out

    # Fronts 0..EARLY-1 run without interleaved backs: the PV of item 0
    # needs the vm tensor, which is still loading while group 0's QK work
    # is already possible; an interleaved (in-order) PE queue would stall.
    n = len(work)
    EARLY = min(5, n)
    LAG = 2
    backlog = list(range(n))   # items whose back phase is pending
    done_front = 0
    for i in range(n):
        take = []
        if i >= EARLY:
            want = len(backlog) - (n - 1 - i)   # drain so last front pairs last back
            want = max(want, 1 if backlog and backlog[0] <= i - LAG else 0)
            for _ in range(min(want, 2)):
                if backlog and backlog[0] <= i - LAG:
                    take.append(backlog.pop(0))
        fr = front_mms(work[i])
        bks = []
        for j in take:
            bks.extend(back_mms(work[j]))
        for thunk in interleave(fr, bks):
            thunk()
        for j in take:
            back_tail(work[j])
        front_tail(work[i])
    while backlog:
        j = backlog.pop(0)
        for thunk in back_mms(work[j]):
            thunk()
        back_tail(work[j])

    for p in reversed(ctxs):
        p.__exit__(None, None, None)
    return pv_first_mms


def _verify_pv_order(nc, pv_first_mms):
    pos = {}
    i = 0
    for bb in nc.m.functions[0].blocks:
        for ins in bb.instructions:
            pos[ins.name] = i
            i += 1
    for first, others in pv_first_mms:
        p0 = pos.get(first)
        if p0 is None:
            return False
        for o in others:
            po = pos.get(o)
            if po is None or po < p0:
                return False
    return True


def _build_kernel(sched: Schedule, safe_pv: bool = False):
    global VM_COLS
    VM_COLS = V_COLS + max(sched.n_mask_cols, 1)
    nc = bacc.Bacc("TRN2", target_bir_lowering=False, debug=False,
                   num_devices=N_CORES, name="sparse_attn")

    qk_in = nc.dram_tensor("qkT", [TS, N_GROUPS, 6, S // N_GROUPS], F16, kind="ExternalInput")
    vm_in = nc.dram_tensor("vm", [TS, VM_COLS], F16, kind="ExternalInput")
    o_out = nc.dram_tensor("o", [TS, N_GROUPS, HEADS_PER_CORE, GROUP_SUBTILES, D + 1],
                           F16, kind="ExternalOutput")
    tensors = (qk_in, vm_in, o_out)

    with tile.TileContext(nc) as tc:
        pv_first_mms = _build_body(nc, tc, sched, tensors, safe_pv=safe_pv)

    nc.compile()
    if not safe_pv and not _verify_pv_order(nc, pv_first_mms):
        return _build_kernel(sched, safe_pv=True)
    return nc


# --------------------------------------------------------------- entry point

_CACHE = {}


def _get_kernel(sched: Schedule):
    key = sched.key()
    if key not in _CACHE:
        _CACHE[key] = _build_kernel(sched)
    return _CACHE[key]


def _shard_inputs(q, k, v, masks_f16, n_mask_cols):
    scale = 1.0 / math.sqrt(D)
    vm_cols = V_COLS + max(n_mask_cols, 1)
    in_maps = []
    for core in range(N_CORES):
        b = core // 4
        m = core % 4
        qk6 = np.empty((TS, 6, S), dtype=np.float16)
        qk6[:, 0:2, :] = k[b, :, 2 * m:2 * m + 2, :].astype(np.float16).transpose(2, 1, 0)
        qk6[:, 2:6, :] = (q[b, :, 4 * m:4 * m + 4, :] * scale).astype(np.float16).transpose(2, 1, 0)
        qkT = np.ascontiguousarray(
            qk6.reshape(TS, 6, N_GROUPS, S // N_GROUPS).transpose(0, 2, 1, 3))
        vc = v[b, :, 2 * m:2 * m + 2, :].astype(np.float16)
        vaug = np.ones((S, 2, D + 1), dtype=np.float16)
        vaug[:, :, :D] = vc
        vaug = vaug.reshape(NT, TS, 2, D + 1).transpose(1, 0, 2, 3)
        vm = np.zeros((TS, vm_cols), dtype=np.float16)
        vm[:, 0:V_COLS] = vaug.reshape(TS, V_COLS)
        vm[:, V_COLS:V_COLS + masks_f16[b].shape[1]] = masks_f16[b]
        in_maps.append({"qkT": qkT, "vm": vm})
    return in_maps


def _host_fix(out, q, k, v, allowed_T, cols):
    """Recompute the given q columns exactly (fp32) and overwrite."""
    if len(cols) == 0:
        return
    scale = 1.0 / math.sqrt(D)
    group = HQ // HKV
    for b in range(B):
        qb = q[b, cols, :, :]                          # [R, HQ, D]
        al = allowed_T[b][:, cols].T                   # [R, S(kv)]
        # logits[r, hq, kv]
        kb = np.repeat(k[b], group, axis=1)            # [S, HQ, D]
        logits = np.einsum("rhd,shd->rhs", qb * scale, kb)
        logits = np.where(al[:, None, :], logits, -np.inf)
        mx = logits.max(axis=-1, keepdims=True)
        e = np.exp(logits - mx)
        p = e / e.sum(axis=-1, keepdims=True)
        vb = np.repeat(v[b], group, axis=1)            # [S, HQ, D]
        out[b, cols, :, :] = np.einsum("rhs,shd->rhd", p, vb)


def kernel(q, k, v, bidirectional_mask, chunk_size):
    q = np.asarray(q, dtype=np.float32)
    k = np.asarray(k, dtype=np.float32)
    v = np.asarray(v, dtype=np.float32)
    chunk = int(np.asarray(chunk_size))

    allowed_T = _allowed_T(bidirectional_mask, chunk)
    sched = Schedule(allowed_T, chunk)
    nc = _get_kernel(sched)

    masks_f16 = [sched.mask_data(allowed_T[b]) for b in range(B)]
    in_maps = _shard_inputs(q, k, v, masks_f16, sched.n_mask_cols)

    res = run_bass_kernel_spmd(nc, in_maps, list(range(N_CORES)))

    out = np.empty((B, S, HQ, D), dtype=np.float32)
    for core in range(N_CORES):
        b = core // 4
        m = core % 4
        oc = res.results[core]["o"]     # [TS, N_GROUPS, 4, GROUP_SUBTILES, D+1]
        oc = oc.transpose(1, 3, 0, 2, 4).reshape(S, HEADS_PER_CORE, D + 1)
        oc = oc.astype(np.float32)
        out[b, :, 4 * m:4 * m + 4, :] = oc[:, :, :D] / oc[:, :, D:]

    _host_fix(out, q, k, v, allowed_T, sched.fix_cols)
    return out
